# revision 4
# baseline (speedup 1.0000x reference)
"""Bundle-adjustment projection kernel for Trainium2 (8 NeuronCores).

v5 minus the constant ones-row: the point stream is 32 three-row groups
([x,y,z] only, 96 rows -- a clean multiple of 32), and the per-(tile,group)
translation terms A3/B3/C3 are applied as per-partition biases instead:
on the Act engine PSUM->SBUF copy for w (activation Identity with bias AP)
and fused into the DVE multiply for u,v (scalar_tensor_tensor
(u + biasA) * rec).  Cuts the dominant X stream by 25%.
"""
import sys
sys.path.insert(0, "/opt/trn_rl_repo")

import numpy as np

FX, FY, CX, CY = 320.0, 320.0, 320.0, 240.0
N_MP, N_KF, M = 200000, 2000, 4000000
N_CORES = 8
SEG = 512
GROUPS = 32                       # 3-row groups per tile
ROWS = 3 * GROUPS                 # 96 contraction rows
T = 34                            # tiles per core (even)
TSUP = T // 2                     # 17 super-tiles
SEGS_CORE = T * GROUPS            # 1088
NSEG_TOT = N_CORES * SEGS_CORE    # 8704

_CACHE = {}


def _build(n_rep=1):
    import concourse.bacc as bacc
    import concourse.mybir as mybir
    import concourse.tile as tile

    f32 = mybir.dt.float32
    f16 = mybir.dt.float16
    Alu = mybir.AluOpType
    Act = mybir.ActivationFunctionType

    nc = bacc.Bacc(None, target_bir_lowering=False)
    x_h = nc.dram_tensor("X", [TSUP * ROWS, 2 * SEG], f16, kind="ExternalInput")
    w_h = nc.dram_tensor("W", [ROWS, T * 128], f16, kind="ExternalInput")
    b_h = nc.dram_tensor("B", [128, T], f32, kind="ExternalInput")
    out_h = nc.dram_tensor("out", [TSUP * 64, 2 * SEG], f16, kind="ExternalOutput")

    with tile.TileContext(nc) as tc:
        with (
            tc.tile_pool(name="const", bufs=1) as constp,
            tc.tile_pool(name="work", bufs=6) as work,
            tc.tile_pool(name="psum", bufs=4, space="PSUM") as psump,
        ):
            wall = constp.tile([ROWS, T * 128], f16)
            nc.sync.dma_start(wall[:], w_h[:])
            ball = constp.tile([128, T], f32)
            nc.sync.dma_start(ball[:], b_h[:])
            for _rep in range(n_rep):
                for s in range(TSUP):
                    xt = work.tile([ROWS, 2 * SEG], f16, tag="x")
                    nc.sync.dma_start(xt[:], x_h[s * ROWS:(s + 1) * ROWS, :])
                    ps = psump.tile([128, 2 * SEG], f32, tag="ps")
                    wsb = work.tile([64, 2 * SEG], f32, tag="wsb")
                    for h in range(2):
                        t = 2 * s + h
                        cs = slice(h * SEG, (h + 1) * SEG)
                        nc.tensor.matmul(
                            out=ps[:, cs],
                            lhsT=wall[:, t * 128:(t + 1) * 128],
                            rhs=xt[:, cs], start=True, stop=True)
                        nc.scalar.activation(
                            wsb[:, cs], ps[64:128, cs], Act.Identity,
                            bias=ball[64:128, t:t + 1], scale=1.0)
                    rec = work.tile([64, 2 * SEG], f32, tag="rec")
                    nc.vector.reciprocal_approx_fast(rec[:], wsb[:])
                    st = work.tile([64, 2 * SEG], f16, tag="st")
                    for h in range(2):
                        t = 2 * s + h
                        cs = slice(h * SEG, (h + 1) * SEG)
                        nc.vector.scalar_tensor_tensor(
                            st[:, cs], ps[0:64, cs], ball[0:64, t:t + 1],
                            rec[:, cs], op0=Alu.add, op1=Alu.mult)
                    nc.sync.dma_start(out_h[s * 64:(s + 1) * 64, :], st[:, :])
    nc.finalize()
    return nc


def _prep_inputs(tMP, tKF, kf_ids, mp_ids, idxKF, idxMP):
    tMP = np.asarray(tMP, np.float32)
    tKF = np.asarray(tKF, np.float32)
    ids_kf = np.searchsorted(np.asarray(idxKF), np.asarray(kf_ids)).astype(np.int64)
    ids_mp = np.searchsorted(np.asarray(idxMP), np.asarray(mp_ids)).astype(np.int64)
    perm = np.argsort(ids_kf, kind="stable")
    kf_s = ids_kf[perm]
    mp_s = ids_mp[perm]

    counts = np.bincount(kf_s, minlength=N_KF)
    nseg = (counts + SEG - 1) // SEG
    NSEG = int(nseg.sum())
    assert NSEG <= NSEG_TOT, f"padded segments {NSEG} exceed capacity {NSEG_TOT}"
    seg_kf = np.full(NSEG_TOT, -1, np.int64)
    seg_kf[:NSEG] = np.repeat(np.arange(N_KF), nseg)

    kf_start_edge = np.concatenate([[0], np.cumsum(counts)])
    kf_first_seg = np.concatenate([[0], np.cumsum(nseg)])
    off = np.arange(M) - kf_start_edge[kf_s]
    seg_e = kf_first_seg[kf_s] + off // SEG
    col_e = off % SEG

    # Point stream: [NSEG_TOT, 3, SEG] fp16 ([x,y,z]; pad slots [0,0,1]).
    Xs = np.zeros((NSEG_TOT, 3, SEG), np.float16)
    Xs[:, 2, :] = 1.0
    Xs[seg_e, :, col_e] = tMP[mp_s].astype(np.float16)

    # Per-kf projection rows (+ pad row: A=B=0, C=[0,0,1,0] -> w=z=1, out 0).
    A = FX * tKF[:, 0, :] + CX * tKF[:, 2, :]
    B = FY * tKF[:, 1, :] + CY * tKF[:, 2, :]
    C = tKF[:, 2, :]
    Aex = np.concatenate([A, np.zeros((1, 4), np.float32)])
    Bex = np.concatenate([B, np.zeros((1, 4), np.float32)])
    Cex = np.concatenate([C, np.array([[0, 0, 1, 0]], np.float32)])
    segA = Aex[seg_kf]
    segB = Bex[seg_kf]
    segC = Cex[seg_kf]

    gidx = np.arange(NSEG_TOT)
    core_ = gidx // SEGS_CORE
    t_ = (gidx // GROUPS) % T
    g_ = gidx % GROUPS
    W = np.zeros((N_CORES, T, ROWS, 128), np.float16)
    Bias = np.zeros((N_CORES, 128, T), np.float32)
    for coefs, coff in ((segA, 0), (segB, 32), (segC, 64), (segC, 96)):
        for c in range(3):
            W[core_, t_, 3 * g_ + c, coff + g_] = coefs[:, c].astype(np.float16)
        Bias[core_, coff + g_, t_] = coefs[:, 3]

    Xr = Xs.reshape(N_CORES, T, ROWS, SEG)
    in_maps = []
    for c in range(N_CORES):
        Xc = Xr[c].reshape(TSUP, 2, ROWS, SEG).transpose(0, 2, 1, 3)
        in_maps.append({
            "X": np.ascontiguousarray(Xc.reshape(TSUP * ROWS, 2 * SEG)),
            "W": np.ascontiguousarray(W[c].transpose(1, 0, 2).reshape(ROWS, T * 128)),
            "B": np.ascontiguousarray(Bias[c]),
        })
    return in_maps, (perm, seg_e, col_e)


def _unshard(outs, meta):
    perm, seg_e, col_e = meta
    core_e = seg_e // SEGS_CORE
    t_e = (seg_e // GROUPS) % T
    g_e = seg_e % GROUPS
    sup = t_e // 2
    colo = (t_e % 2) * SEG + col_e
    stacked = np.stack(outs).astype(np.float32)  # [8, TSUP*64, 1024]
    res = np.empty((M, 2), np.float32)
    res[perm, 0] = stacked[core_e, sup * 64 + g_e, colo]
    res[perm, 1] = stacked[core_e, sup * 64 + 32 + g_e, colo]
    return res


def kernel(tMP, tKF, kf_ids, mp_ids, idxKF, idxMP):
    from concourse.bass_utils import run_bass_kernel_spmd

    if "nc" not in _CACHE:
        _CACHE["nc"] = _build()
    nc = _CACHE["nc"]
    in_maps, meta = _prep_inputs(tMP, tKF, kf_ids, mp_ids, idxKF, idxMP)
    res = run_bass_kernel_spmd(nc, in_maps, core_ids=list(range(N_CORES)))
    outs = [res.results[i]["out"] for i in range(N_CORES)]
    return _unshard(outs, meta)
